# revision 24
# baseline (speedup 1.0000x reference)
"""BagAttention Trainium2 kernel (8-core SPMD, Bass/Tile).

Computes, per bag b (rows given by cumulative offsets `scope`):
    l_i   = d**-0.5 * (x_i @ w)
    attn_i = softmax_{i in bag}(l_i)
    out_b = sum_i attn_i * x_i
Returns (bag_logits [NB, D], attn [TOTAL]) like the reference.

Sharding: data-parallel over bags — core c owns bags [c*NB/8,(c+1)*NB/8)
and their contiguous rows of x. No cross-core comms.

Numerics note: the reference subtracts the per-bag max before exp.
Softmax is shift-invariant, and with attn_w ~ N(0, 0.01^2), |l| <~ 0.05,
so we skip the max (clamping l at 80 as an overflow guard for padded
rows). This only perturbs results at the fp32 rounding level.

Per-core structure (P = 128 partitions):
  - x is processed in 128-row tiles; rows live on partitions.
  - bags are processed in groups of 128; group bag-sums live on the
    partitions of PSUM accumulators.
  - per tile: one fused DVE tensor_tensor_reduce gives l = sum(x*w) per
    row; ACT exp gives e; a [128 rows x 128 bags] matrix
    lhsT_e[p, c] = e_p * (bagcol_p == c) is built in one DVE
    tensor_scalar (is_equal then mult against a column-iota constant).
    TensorE then accumulates numer[bag, :] += lhsT_e.T @ x_tile and
    denom[bag] += lhsT_e.T @ ones into the group's PSUM.
  - per group: r = 1/denom (DVE), out = numer * r via ScalarE
    copy-with-scale (PSUM -> SBUF), DMA out. attn_i = e_i * r_bag(i) via
    a tiny matmul with the transposed mask (gathers r across partitions)
    then ScalarE mul; attn is emitted in a [128, ntiles] layout
    (transposed back on host).
"""

import os
import sys

import numpy as np

sys.path.insert(0, "/opt/trn_rl_repo")

import concourse.bacc as bacc  # noqa: E402
import concourse.bass as bass  # noqa: E402
import concourse.tile as tile  # noqa: E402
from concourse import mybir  # noqa: E402
from concourse.bass_utils import run_bass_kernel_spmd  # noqa: E402

P = 128  # partitions
NCORES = 8
DMA_TILES = 4  # x tiles per dma_start (512 rows, ~1.5 MiB)
CLAMP = 80.0  # overflow guard for exp on (zero-padded) garbage rows

# test.py introspection: last run's BassKernelResults
last_results = None

_PROGRAM_CACHE = {}


# ----------------------------------------------------------------------------
# Host-side planning: derive the per-core tile/group/item structure from scope.
# ----------------------------------------------------------------------------

class _Item:
    __slots__ = ("t", "g", "slot", "c0", "nb", "bagrel", "mt", "start", "stop",
                 "p0", "p1")


def _plan_core(scope: np.ndarray, core: int):
    nb_total = scope.shape[0]
    assert nb_total % NCORES == 0
    bpc = nb_total // NCORES
    assert bpc % P == 0
    bag_lo = core * bpc
    row_lo = 0 if bag_lo == 0 else int(scope[bag_lo - 1])
    row_hi = int(scope[bag_lo + bpc - 1])
    ends = scope[bag_lo:bag_lo + bpc].astype(np.int64) - row_lo
    sizes = np.diff(np.concatenate([[0], ends]))
    assert (sizes >= 0).all(), "scope must be nondecreasing"
    R = int(ends[-1])
    assert R == row_hi - row_lo
    T = max(1, -(-R // P))
    nchunks = -(-T // DMA_TILES)
    T = nchunks * DMA_TILES
    R_pad = T * P
    # local bag index per local row; padded rows get -1
    bag_of_row = np.full(R_pad, -1, dtype=np.int64)
    bag_of_row[:R] = np.repeat(np.arange(bpc), sizes)
    G = bpc // P

    items = []
    for t in range(T):
        rows = bag_of_row[t * P:(t + 1) * P]
        present = rows[rows >= 0]
        if present.size == 0:
            continue
        for slot, g in enumerate(sorted(set((present // P).tolist()))):
            it = _Item()
            it.t, it.g, it.slot = t, int(g), slot
            sel = (rows >= 0) & (rows // P == g)
            cols = np.where(sel, rows - g * P, -1)
            it.bagrel = cols.astype(np.float32)
            cs = cols[sel]
            it.c0 = int(cs.min())
            it.nb = int(cs.max()) - it.c0 + 1
            pp = np.nonzero(sel)[0]
            it.p0, it.p1 = int(pp.min()), int(pp.max()) + 1
            assert it.p1 - it.p0 == pp.size, "item rows must be contiguous"
            items.append(it)

    # start/stop flags per group (PSUM accumulation chain order = emission)
    by_group = {}
    for it in items:
        by_group.setdefault(it.g, []).append(it)
    for g, gi in by_group.items():
        for it in gi:
            it.start = it is gi[0]
            it.stop = it is gi[-1]
    nslots = max(it.slot for it in items) + 1
    return dict(R=R, R_pad=R_pad, T=T, G=G, bpc=bpc, row_lo=row_lo,
                row_hi=row_hi, items=items, by_group=by_group, nslots=nslots)


def _structure_sig(plan):
    return (plan["R_pad"], plan["T"], plan["G"], plan["nslots"],
            tuple((it.t, it.g, it.slot, it.c0, it.nb, it.start, it.stop,
                   it.p0, it.p1)
                  for it in plan["items"]))


# ----------------------------------------------------------------------------
# Program construction (single SPMD program shared by all cores).
# ----------------------------------------------------------------------------

def _build_program(plan, D):
    f32 = mybir.dt.float32
    T, G, bpc = plan["T"], plan["G"], plan["bpc"]
    items = plan["items"]
    nitems = len(items)

    nc = bacc.Bacc("TRN2", num_devices=NCORES)
    x_t = nc.dram_tensor("x", [plan["R_pad"], D], f32, kind="ExternalInput")
    w_t = nc.dram_tensor("w", [D], f32, kind="ExternalInput")
    winv_t = nc.dram_tensor("winv", [D], f32, kind="ExternalInput")
    iota_t = nc.dram_tensor("colio", [P, P], f32, kind="ExternalInput")
    ident_t = nc.dram_tensor("ident", [P, P], f32, kind="ExternalInput")
    br_t = nc.dram_tensor("bagrel", [P, nitems], f32, kind="ExternalInput")
    out_t = nc.dram_tensor("bag_logits", [bpc, D], f32, kind="ExternalOutput")
    # attn in [tile, slot, partition] layout; each (tile, slot) gets its own
    # 512B DRAM line so no two DMAs touch the same line (sub-512B DMA writes
    # are read-modify-write -> concurrent partial writes to one line race).
    # Non-item rows of each column are exact zeros (matmul output), so the
    # host just sums the slots per tile.
    attn_t = nc.dram_tensor("attn", [T, 2, P], f32, kind="ExternalOutput")

    x_re = x_t.ap().rearrange("(j p) d -> p j d", p=P)
    scale = float(D) ** -0.5
    n0 = min(512, D)  # matmul free-dim split (one PSUM bank per matmul)

    with tile.TileContext(nc) as tc:
        with (
            tc.tile_pool(name="singles", bufs=1) as singles,
            tc.tile_pool(name="xchunks", bufs=3) as xpool,
            tc.tile_pool(name="work", bufs=3) as work,
            tc.tile_pool(name="ebuf", bufs=2 * DMA_TILES + 8) as ebuf,
            tc.tile_pool(name="lhst_pool", bufs=16) as lhst_pool,
            tc.tile_pool(name="metbuf", bufs=3) as metbuf,
            tc.tile_pool(name="outbuf", bufs=2) as outbuf,
            tc.tile_pool(name="ps_num", bufs=2, space="PSUM") as ps_num,
            tc.tile_pool(name="ps_den", bufs=2, space="PSUM") as ps_den,
            tc.tile_pool(name="ps_tr", bufs=2, space="PSUM") as ps_tr,
        ):
            def bcast_load(tile_ap, src_t):
                src = src_t.ap()
                nc.sync.dma_start(
                    out=tile_ap,
                    in_=bass.AP(tensor=src.tensor, offset=src.offset,
                                ap=[[0, P]] + list(src.ap)),
                )

            w_sb = singles.tile([P, D], f32)
            bcast_load(w_sb, w_t)
            winv_sb = singles.tile([P, D], f32)
            bcast_load(winv_sb, winv_t)
            colio_sb = singles.tile([P, P], f32)
            nc.sync.dma_start(out=colio_sb, in_=iota_t.ap())
            ident_sb = singles.tile([P, P], f32)
            nc.sync.dma_start(out=ident_sb, in_=ident_t.ap())
            br_all = singles.tile([P, nitems], f32)
            nc.sync.dma_start(out=br_all, in_=br_t.ap())
            ones_sb = singles.tile([P, 1], f32)
            nc.vector.memset(ones_sb, 1.0)

            x_chunks = {}   # chunk index -> tile
            e_tiles = {}    # tile index -> e [P,1]
            xw_tiles = {}   # tile index -> xw = x*w [P,D] (PE consumes this)
            seen_tiles = set()

            def do_tile(t):
                # logits + exp for tile t (first time t is touched).
                # The x chunk is read ONLY by the DVE (tensor_tensor_reduce):
                # keeping a single reader engine keeps the x-load DMA at <=2
                # sync waits (walrus DMA instruction limit). The PE consumes
                # the product xw instead; the final per-group result is
                # divided by w again (numerically benign: every summand in
                # numer[:, d] carries the same factor w_d).
                ci = t // DMA_TILES
                if ci not in x_chunks:
                    x_chunk = xpool.tile([P, DMA_TILES, D], f32, tag="xchunk")
                    # SWDGE (gpsimd): Q7 executes sync waits in software, so
                    # the slot-reuse WAR+WAW deps don't hit the 1-wait limit
                    # of HWDGE-lowered DMA instructions.
                    nc.gpsimd.dma_start(
                        out=x_chunk,
                        in_=x_re[:, ci * DMA_TILES:(ci + 1) * DMA_TILES, :],
                    )
                    x_chunks[ci] = x_chunk
                x_sub = x_chunks[ci][:, t % DMA_TILES, :]
                xw = work.tile([P, D], f32, tag="xw")
                l_sb = work.tile([P, 1], f32, tag="l")
                # fused dot product: xw = x*w, l = sum(xw) in ONE DVE pass
                # (affine_mul_reduce is the table-registered custom DVE op;
                # raw tensor_tensor_reduce lacks the ucode table and dies on HW)
                nc.vector.affine_mul_reduce(
                    out=xw, accum_out=l_sb, in0=x_sub, in1=w_sb,
                    scale=1.0, bias=0.0,
                )
                nc.vector.tensor_scalar(
                    out=l_sb, in0=l_sb, scalar1=scale, scalar2=CLAMP,
                    op0=mybir.AluOpType.mult, op1=mybir.AluOpType.min,
                )
                e_sb = ebuf.tile([P, 1], f32, tag="e")
                nc.scalar.activation(out=e_sb, in_=l_sb,
                                     func=mybir.ActivationFunctionType.Exp)
                e_tiles[t] = e_sb
                xw_tiles[t] = xw

            for g in range(G):
                gitems = plan["by_group"].get(g, [])
                num_ps = ps_num.tile([P, D], f32, tag="num")
                den_ps = ps_den.tile([P, 1], f32, tag="den")
                lhst_items = {}
                for it in gitems:
                    if it.t not in seen_tiles:
                        seen_tiles.add(it.t)
                        do_tile(it.t)
                    i = items.index(it)
                    xw = xw_tiles[it.t]
                    lhsT_e = lhst_pool.tile([P, P], f32, tag="lhst")
                    nc.vector.tensor_scalar(
                        out=lhsT_e, in0=colio_sb, scalar1=br_all[:, i:i + 1],
                        scalar2=e_tiles[it.t],
                        op0=mybir.AluOpType.is_equal, op1=mybir.AluOpType.mult,
                    )
                    lhst_items[i] = lhsT_e
                    nc.tensor.matmul(num_ps[:, 0:n0], lhsT_e, xw[:, 0:n0],
                                     start=it.start, stop=it.stop)
                    if D > n0:
                        nc.tensor.matmul(num_ps[:, n0:D], lhsT_e, xw[:, n0:D],
                                         start=it.start, stop=it.stop)
                    nc.tensor.matmul(den_ps, lhsT_e, ones_sb,
                                     start=it.start, stop=it.stop)

                # ---- group reduction done: normalize + outputs ----
                # num_ps holds sum_i e_i * (w_d * x_id); scale by r_b = 1/denom
                # (per-partition, ScalarE) then by 1/w_d (per-column, DVE).
                r_sb = work.tile([P, 1], f32, tag="r")
                nc.vector.reciprocal(r_sb, den_ps)
                nrm_sb = outbuf.tile([P, D], f32, tag="nrm")
                nc.scalar.mul(nrm_sb, num_ps, mul=r_sb)
                out_sb = outbuf.tile([P, D], f32, tag="out")
                nc.vector.tensor_mul(out_sb, nrm_sb, winv_sb)
                nc.gpsimd.dma_start(out=out_t.ap()[g * P:(g + 1) * P, :], in_=out_sb)

                # attn_i = e_i * r_bag(i): transpose the e-scaled mask on the
                # PE, then one matmul gathers r across partitions.
                for it in gitems:
                    i = items.index(it)
                    met_ps = ps_tr.tile([P, P], f32, tag="tr")
                    nc.tensor.transpose(met_ps, lhst_items[i], ident_sb)
                    met_sb = metbuf.tile([P, P], f32, tag="met")
                    nc.scalar.copy(met_sb, met_ps)
                    attn_ps = ps_tr.tile([P, 1], f32, tag="tr")
                    nc.tensor.matmul(attn_ps, met_sb, r_sb, start=True, stop=True)
                    attn_sb = work.tile([P, 1], f32, tag="attn")
                    nc.scalar.copy(attn_sb, attn_ps)
                    nc.gpsimd.dma_start(
                        out=attn_t.ap()[it.t, it.slot, :],
                        in_=attn_sb,
                    )

    nc.finalize()
    return nc


# ----------------------------------------------------------------------------
# Entry point
# ----------------------------------------------------------------------------

def kernel(x: np.ndarray, attn_w: np.ndarray, scope: np.ndarray):
    global last_results
    x = np.ascontiguousarray(np.asarray(x, dtype=np.float32))
    w = np.ascontiguousarray(np.asarray(attn_w, dtype=np.float32))
    scope_np = np.asarray(scope).astype(np.int64)
    total, D = x.shape
    nb_total = scope_np.shape[0]

    plans = [_plan_core(scope_np, c) for c in range(NCORES)]
    sig0 = _structure_sig(plans[0])
    for c in range(1, NCORES):
        assert _structure_sig(plans[c]) == sig0, (
            "per-core structures differ; SPMD program requires symmetric "
            "bag layout across cores"
        )

    key = (sig0, D)
    if key not in _PROGRAM_CACHE:
        _PROGRAM_CACHE[key] = _build_program(plans[0], D)
    nc = _PROGRAM_CACHE[key]

    assert np.abs(w).min() > 1e-30, (
        "attn_w contains (near-)zero entries; the w-division trick needs w != 0"
    )
    winv = (1.0 / w).astype(np.float32)
    colio = np.broadcast_to(np.arange(P, dtype=np.float32), (P, P)).copy()
    in_maps = []
    for c, pl in enumerate(plans):
        xc = x[pl["row_lo"]:pl["row_hi"]]
        if xc.shape[0] < pl["R_pad"]:
            xc = np.concatenate(
                [xc, np.zeros((pl["R_pad"] - xc.shape[0], D), np.float32)])
        bagrel = np.stack([it.bagrel for it in pl["items"]], axis=1)  # [P, nitems]
        in_maps.append({
            "x": np.ascontiguousarray(xc),
            "w": w,
            "winv": winv,
            "colio": colio,
            "ident": np.eye(P, dtype=np.float32),
            "bagrel": np.ascontiguousarray(bagrel),
        })

    trace = bool(os.environ.get("BASS_TRACE"))
    last_results = run_bass_kernel_spmd(
        nc, in_maps, core_ids=list(range(NCORES)), trace=trace)

    bag_logits = np.concatenate(
        [last_results.results[c]["bag_logits"] for c in range(NCORES)])
    attn_parts = []
    for c, pl in enumerate(plans):
        arr = last_results.results[c]["attn"]  # [T, 2, P]
        acc = arr[:, 0, :].copy()
        for it in pl["items"]:
            if it.slot == 1:
                acc[it.t] += arr[it.t, 1]
        attn_parts.append(acc.reshape(-1)[:pl["R"]])
    attn = np.concatenate(attn_parts)

    # empty bags: reference yields exactly 0 for their pooled output
    sizes = np.diff(np.concatenate([[0], scope_np]))
    if (sizes == 0).any():
        bag_logits[sizes == 0] = 0.0
    return bag_logits.astype(np.float32), attn.astype(np.float32)


# revision 29
# speedup vs baseline: 6.1824x; 6.1824x over previous
"""BagAttention Trainium2 kernel (8-core SPMD, Bass/Tile).

Computes, per bag b (rows given by cumulative offsets `scope`):
    l_i   = d**-0.5 * (x_i @ w)
    attn_i = softmax_{i in bag}(l_i)
    out_b = sum_i attn_i * x_i
Returns (bag_logits [NB, D], attn [TOTAL]) like the reference.

Sharding: data-parallel over bags — core c owns bags [c*NB/8,(c+1)*NB/8)
and their contiguous rows of x. No cross-core comms.

Numerics note: the reference subtracts the per-bag max before exp.
Softmax is shift-invariant, and with attn_w ~ N(0, 0.01^2), |l| <~ 0.05,
so we skip the max (clamping l at 80 as an overflow guard for padded
rows). This only perturbs results at the fp32 rounding level.

Per-core structure (P = 128 partitions):
  - x is processed in 128-row tiles; rows live on partitions.
  - bags are processed in groups of 128; group bag-sums live on the
    partitions of PSUM accumulators.
  - per tile: one fused DVE tensor_tensor_reduce gives l = sum(x*w) per
    row; ACT exp gives e; a [128 rows x 128 bags] matrix
    lhsT_e[p, c] = e_p * (bagcol_p == c) is built in one DVE
    tensor_scalar (is_equal then mult against a column-iota constant).
    TensorE then accumulates numer[bag, :] += lhsT_e.T @ x_tile and
    denom[bag] += lhsT_e.T @ ones into the group's PSUM.
  - per group: r = 1/denom (DVE), out = numer * r via ScalarE
    copy-with-scale (PSUM -> SBUF), DMA out. attn_i = e_i * r_bag(i) via
    a tiny matmul with the transposed mask (gathers r across partitions)
    then ScalarE mul; attn is emitted in a [128, ntiles] layout
    (transposed back on host).
"""

import os
import sys

import numpy as np

sys.path.insert(0, "/opt/trn_rl_repo")

import concourse.bacc as bacc  # noqa: E402
import concourse.bass as bass  # noqa: E402
import concourse.tile as tile  # noqa: E402
from concourse import mybir  # noqa: E402
from concourse.bass_utils import run_bass_kernel_spmd  # noqa: E402

P = 128  # partitions
NCORES = 8
DMA_TILES = 4  # x tiles per dma_start (512 rows, ~1.5 MiB)
CLAMP = 80.0  # overflow guard for exp on (zero-padded) garbage rows

# test.py introspection: last run's BassKernelResults
last_results = None

_PROGRAM_CACHE = {}


# ----------------------------------------------------------------------------
# Host-side planning: derive the per-core tile/group/item structure from scope.
# ----------------------------------------------------------------------------

class _Item:
    __slots__ = ("t", "g", "slot", "c0", "nb", "bagrel", "mt", "start", "stop",
                 "p0", "p1")


def _plan_core(scope: np.ndarray, core: int):
    nb_total = scope.shape[0]
    assert nb_total % NCORES == 0
    bpc = nb_total // NCORES
    assert bpc % P == 0
    bag_lo = core * bpc
    row_lo = 0 if bag_lo == 0 else int(scope[bag_lo - 1])
    row_hi = int(scope[bag_lo + bpc - 1])
    ends = scope[bag_lo:bag_lo + bpc].astype(np.int64) - row_lo
    sizes = np.diff(np.concatenate([[0], ends]))
    assert (sizes >= 0).all(), "scope must be nondecreasing"
    R = int(ends[-1])
    assert R == row_hi - row_lo
    T = max(1, -(-R // P))
    nchunks = -(-T // DMA_TILES)
    T = nchunks * DMA_TILES
    R_pad = T * P
    # local bag index per local row; padded rows get -1
    bag_of_row = np.full(R_pad, -1, dtype=np.int64)
    bag_of_row[:R] = np.repeat(np.arange(bpc), sizes)
    G = bpc // P

    items = []
    for t in range(T):
        rows = bag_of_row[t * P:(t + 1) * P]
        present = rows[rows >= 0]
        if present.size == 0:
            continue
        for slot, g in enumerate(sorted(set((present // P).tolist()))):
            it = _Item()
            it.t, it.g, it.slot = t, int(g), slot
            sel = (rows >= 0) & (rows // P == g)
            cols = np.where(sel, rows - g * P, -1)
            it.bagrel = cols.astype(np.float32)
            cs = cols[sel]
            it.c0 = int(cs.min())
            it.nb = int(cs.max()) - it.c0 + 1
            pp = np.nonzero(sel)[0]
            it.p0, it.p1 = int(pp.min()), int(pp.max()) + 1
            assert it.p1 - it.p0 == pp.size, "item rows must be contiguous"
            items.append(it)

    # start/stop flags per group (PSUM accumulation chain order = emission)
    by_group = {}
    for it in items:
        by_group.setdefault(it.g, []).append(it)
    for g, gi in by_group.items():
        for it in gi:
            it.start = it is gi[0]
            it.stop = it is gi[-1]
    nslots = max(it.slot for it in items) + 1
    return dict(R=R, R_pad=R_pad, T=T, G=G, bpc=bpc, row_lo=row_lo,
                row_hi=row_hi, items=items, by_group=by_group, nslots=nslots)


def _structure_sig(plan):
    return (plan["R_pad"], plan["T"], plan["G"], plan["nslots"],
            tuple((it.t, it.g, it.slot, it.c0, it.nb, it.start, it.stop,
                   it.p0, it.p1)
                  for it in plan["items"]))


# ----------------------------------------------------------------------------
# Program construction (single SPMD program shared by all cores).
# ----------------------------------------------------------------------------

def _build_program(plan, D, reps=1, emit_attn=True):
    f32 = mybir.dt.float32
    T, G, bpc = plan["T"], plan["G"], plan["bpc"]
    items = plan["items"]
    nitems = len(items)

    nc = bacc.Bacc("TRN2", num_devices=NCORES)
    x_t = nc.dram_tensor("x", [plan["R_pad"], D], f32, kind="ExternalInput")
    w_t = nc.dram_tensor("w", [D], f32, kind="ExternalInput")
    winv_t = nc.dram_tensor("winv", [D], f32, kind="ExternalInput")
    iota_t = nc.dram_tensor("colio", [P, P], f32, kind="ExternalInput")
    ident_t = nc.dram_tensor("ident", [P, P], f32, kind="ExternalInput")
    br_t = nc.dram_tensor("bagrel", [P, nitems], f32, kind="ExternalInput")
    out_t = nc.dram_tensor("bag_logits", [bpc, D], f32, kind="ExternalOutput")
    # attn in [tile, slot, partition] layout; each (tile, slot) gets its own
    # 512B DRAM line so no two DMAs touch the same line (sub-512B DMA writes
    # are read-modify-write -> concurrent partial writes to one line race).
    # Non-item rows of each column are exact zeros (matmul output), so the
    # host just sums the slots per tile.
    attn_t = nc.dram_tensor("attn", [T, 2, P], f32, kind="ExternalOutput")

    x_re = x_t.ap().rearrange("(j p) d -> p j d", p=P)
    scale = float(D) ** -0.5
    n0 = min(512, D)  # matmul free-dim split (one PSUM bank per matmul)

    with tile.TileContext(nc) as tc:
        with (
            tc.tile_pool(name="singles", bufs=1) as singles,
            tc.tile_pool(name="xchunks", bufs=3) as xpool,
            tc.tile_pool(name="work", bufs=3) as work,
            tc.tile_pool(name="ebuf", bufs=2 * DMA_TILES + 8) as ebuf,
            tc.tile_pool(name="lhst_pool", bufs=16) as lhst_pool,
            tc.tile_pool(name="metbuf", bufs=6) as metbuf,
            tc.tile_pool(name="outbuf", bufs=2) as outbuf,
            tc.tile_pool(name="ps_num", bufs=2, space="PSUM") as ps_num,
            tc.tile_pool(name="ps_den", bufs=2, space="PSUM") as ps_den,
            tc.tile_pool(name="ps_tr", bufs=2, space="PSUM") as ps_tr,
        ):
            def bcast_load(tile_ap, src_t):
                src = src_t.ap()
                nc.sync.dma_start(
                    out=tile_ap,
                    in_=bass.AP(tensor=src.tensor, offset=src.offset,
                                ap=[[0, P]] + list(src.ap)),
                )

            w_sb = singles.tile([P, D], f32)
            bcast_load(w_sb, w_t)
            winv_sb = singles.tile([P, D], f32)
            bcast_load(winv_sb, winv_t)
            colio_sb = singles.tile([P, P], f32)
            nc.sync.dma_start(out=colio_sb, in_=iota_t.ap())
            ident_sb = singles.tile([P, P], f32)
            nc.sync.dma_start(out=ident_sb, in_=ident_t.ap())
            br_all = singles.tile([P, nitems], f32)
            nc.sync.dma_start(out=br_all, in_=br_t.ap())
            ones_sb = singles.tile([P, 1], f32)
            nc.vector.memset(ones_sb, 1.0)

            x_chunks = {}   # chunk index -> tile
            e_tiles = {}    # tile index -> e [P,1]
            xw_tiles = {}   # tile index -> xw = x*w [P,D] (PE consumes this)
            seen_tiles = set()
            rep_state = [x_chunks, e_tiles, xw_tiles, seen_tiles]

            def do_tile(t):
                # logits + exp for tile t (first time t is touched).
                # The x chunk is read ONLY by the DVE (tensor_tensor_reduce):
                # keeping a single reader engine keeps the x-load DMA at <=2
                # sync waits (walrus DMA instruction limit). The PE consumes
                # the product xw instead; the final per-group result is
                # divided by w again (numerically benign: every summand in
                # numer[:, d] carries the same factor w_d).
                ci = t // DMA_TILES
                if ci not in x_chunks:
                    x_chunk = xpool.tile([P, DMA_TILES, D], f32, tag="xchunk")
                    # SWDGE (gpsimd): Q7 executes sync waits in software, so
                    # the slot-reuse WAR+WAW deps don't hit the 1-wait limit
                    # of HWDGE-lowered DMA instructions.
                    nc.gpsimd.dma_start(
                        out=x_chunk,
                        in_=x_re[:, ci * DMA_TILES:(ci + 1) * DMA_TILES, :],
                    )
                    x_chunks[ci] = x_chunk
                x_sub = x_chunks[ci][:, t % DMA_TILES, :]
                xw = work.tile([P, D], f32, tag="xw")
                l_sb = work.tile([P, 1], f32, tag="l")
                # fused dot product: xw = x*w, l = sum(xw) in ONE DVE pass
                # (affine_mul_reduce is the table-registered custom DVE op;
                # raw tensor_tensor_reduce lacks the ucode table and dies on HW)
                nc.vector.affine_mul_reduce(
                    out=xw, accum_out=l_sb, in0=x_sub, in1=w_sb,
                    scale=1.0, bias=0.0,
                )
                nc.vector.tensor_scalar(
                    out=l_sb, in0=l_sb, scalar1=scale, scalar2=CLAMP,
                    op0=mybir.AluOpType.mult, op1=mybir.AluOpType.min,
                )
                e_sb = ebuf.tile([P, 1], f32, tag="e")
                nc.scalar.activation(out=e_sb, in_=l_sb,
                                     func=mybir.ActivationFunctionType.Exp)
                e_tiles[t] = e_sb
                xw_tiles[t] = xw

            for _rep in range(reps):
              for s in rep_state:
                  s.clear()
              for g in range(G):
                gitems = plan["by_group"].get(g, [])
                num_ps = ps_num.tile([P, D], f32, tag="num")
                den_ps = ps_den.tile([P, 1], f32, tag="den")
                lhst_items = {}
                for it in gitems:
                    if it.t not in seen_tiles:
                        seen_tiles.add(it.t)
                        do_tile(it.t)
                    i = items.index(it)
                    xw = xw_tiles[it.t]
                    lhsT_e = lhst_pool.tile([P, P], f32, tag="lhst")
                    nc.vector.tensor_scalar(
                        out=lhsT_e, in0=colio_sb, scalar1=br_all[:, i:i + 1],
                        scalar2=e_tiles[it.t],
                        op0=mybir.AluOpType.is_equal, op1=mybir.AluOpType.mult,
                    )
                    lhst_items[i] = lhsT_e
                    nc.tensor.matmul(num_ps[:, 0:n0], lhsT_e, xw[:, 0:n0],
                                     start=it.start, stop=it.stop)
                    if D > n0:
                        nc.tensor.matmul(num_ps[:, n0:D], lhsT_e, xw[:, n0:D],
                                         start=it.start, stop=it.stop)
                    nc.tensor.matmul(den_ps, lhsT_e, ones_sb,
                                     start=it.start, stop=it.stop)

                # ---- group reduction done: normalize + outputs ----
                # num_ps holds sum_i e_i * (w_d * x_id); scale by r_b = 1/denom
                # (per-partition, ScalarE) then by 1/w_d (per-column, DVE).
                r_sb = work.tile([P, 1], f32, tag="r")
                nc.vector.reciprocal(r_sb, den_ps)
                nrm_sb = outbuf.tile([P, D], f32, tag="nrm")
                nc.scalar.mul(nrm_sb, num_ps, mul=r_sb)
                out_sb = outbuf.tile([P, D], f32, tag="out")
                nc.vector.tensor_mul(out_sb, nrm_sb, winv_sb)
                nc.gpsimd.dma_start(out=out_t.ap()[g * P:(g + 1) * P, :], in_=out_sb)

                # attn_i = e_i * r_bag(i): transpose the e-scaled mask on the
                # PE, then one matmul gathers r across partitions.
                for it in (gitems if emit_attn else []):
                    i = items.index(it)
                    met_ps = ps_tr.tile([P, P], f32, tag="tr")
                    nc.tensor.transpose(met_ps, lhst_items[i], ident_sb)
                    met_sb = metbuf.tile([P, P], f32, tag="met")
                    nc.scalar.copy(met_sb, met_ps)
                    attn_ps = ps_tr.tile([P, 1], f32, tag="tr")
                    nc.tensor.matmul(attn_ps, met_sb, r_sb, start=True, stop=True)
                    attn_sb = work.tile([P, 1], f32, tag="attn")
                    nc.scalar.copy(attn_sb, attn_ps)
                    nc.gpsimd.dma_start(
                        out=attn_t.ap()[it.t, it.slot, :],
                        in_=attn_sb,
                    )

    nc.finalize()
    return nc


# ----------------------------------------------------------------------------
# Entry point
# ----------------------------------------------------------------------------

def kernel(x: np.ndarray, attn_w: np.ndarray, scope: np.ndarray):
    global last_results
    x = np.ascontiguousarray(np.asarray(x, dtype=np.float32))
    w = np.ascontiguousarray(np.asarray(attn_w, dtype=np.float32))
    scope_np = np.asarray(scope).astype(np.int64)
    total, D = x.shape
    nb_total = scope_np.shape[0]

    plans = [_plan_core(scope_np, c) for c in range(NCORES)]
    sig0 = _structure_sig(plans[0])
    for c in range(1, NCORES):
        assert _structure_sig(plans[c]) == sig0, (
            "per-core structures differ; SPMD program requires symmetric "
            "bag layout across cores"
        )

    key = (sig0, D)
    if key not in _PROGRAM_CACHE:
        _PROGRAM_CACHE[key] = _build_program(plans[0], D)
    nc = _PROGRAM_CACHE[key]

    assert np.abs(w).min() > 1e-30, (
        "attn_w contains (near-)zero entries; the w-division trick needs w != 0"
    )
    winv = (1.0 / w).astype(np.float32)
    colio = np.broadcast_to(np.arange(P, dtype=np.float32), (P, P)).copy()
    in_maps = []
    for c, pl in enumerate(plans):
        xc = x[pl["row_lo"]:pl["row_hi"]]
        if xc.shape[0] < pl["R_pad"]:
            xc = np.concatenate(
                [xc, np.zeros((pl["R_pad"] - xc.shape[0], D), np.float32)])
        bagrel = np.stack([it.bagrel for it in pl["items"]], axis=1)  # [P, nitems]
        in_maps.append({
            "x": np.ascontiguousarray(xc),
            "w": w,
            "winv": winv,
            "colio": colio,
            "ident": np.eye(P, dtype=np.float32),
            "bagrel": np.ascontiguousarray(bagrel),
        })

    trace = bool(os.environ.get("BASS_TRACE"))
    last_results = run_bass_kernel_spmd(
        nc, in_maps, core_ids=list(range(NCORES)), trace=trace)

    bag_logits = np.concatenate(
        [last_results.results[c]["bag_logits"] for c in range(NCORES)])
    attn_parts = []
    for c, pl in enumerate(plans):
        arr = last_results.results[c]["attn"]  # [T, 2, P]
        acc = arr[:, 0, :].copy()
        for it in pl["items"]:
            if it.slot == 1:
                acc[it.t] += arr[it.t, 1]
        attn_parts.append(acc.reshape(-1)[:pl["R"]])
    attn = np.concatenate(attn_parts)

    # empty bags: reference yields exactly 0 for their pooled output
    sizes = np.diff(np.concatenate([[0], scope_np]))
    if (sizes == 0).any():
        bag_logits[sizes == 0] = 0.0
    return bag_logits.astype(np.float32), attn.astype(np.float32)
